# revision 3
# baseline (speedup 1.0000x reference)
"""EMA Vector-Quantizer Trainium2 kernel.

Data-parallel over the N token axis across 8 NeuronCores. Each core:
  - computes squared-L2 argmin over the K=4096 codebook for its 4096 tokens
    (fp16 2-way-split matmuls: exact products -> fp32-grade distances),
  - builds one-hot rows, gathers codebook vectors (indirect DMA),
  - accumulates per-shard cluster counts + embed sums (one-hot matmuls),
  - ReduceScatters counts/embed_sum and AllReduces (loss, n) across cores,
  - applies the EMA buffer update on its K/8 codebook slice.
Host only shards/concats, transposes layouts, and casts dtypes.
"""

import numpy as np

import concourse.bass as bass
import concourse.bacc as bacc
import concourse.bass_isa as bass_isa
import concourse.mybir as mybir
import concourse.tile as tile
from concourse.bass_utils import run_bass_kernel_spmd

F32 = mybir.dt.float32
F16 = mybir.dt.float16
I32 = mybir.dt.int32
U32 = mybir.dt.uint32

DECAY = 0.99
EPS = 1e-05
COMMITMENT_COST = 0.25
N, D, K = 32768, 512, 4096
NCORES = 8
NLOC = N // NCORES          # 4096 tokens per core
P = 128
TILES = NLOC // P           # 32 token tiles per core
GRP = 8                     # token tiles per embed group
NGRP = TILES // GRP         # 4
KSLC = K // NCORES          # 512 codes per core for the EMA update
CB = 512                    # code block (matmul moving free dim)
HALF = K // 2               # 2048
DC = D // P                 # 4 contraction chunks
KT = K // P                 # 32 code tiles

_CACHE = {}


def _build_nc():
    nc = bacc.Bacc("TRN2", target_bir_lowering=False, debug=False,
                   num_devices=NCORES)

    din = {}
    for name, shape, dt in [
        ("xh", [D, NLOC], F16),       # fp16 hi of x_shard.T
        ("xl", [D, NLOC], F16),       # fp16 lo of x_shard.T
        ("x16", [NLOC, D], F16),      # fp16 of x_shard
        ("Eh", [D, K], F16),          # fp16 hi of 2*embedding
        ("El", [D, K], F16),          # fp16 lo of 2*embedding
        ("esq", [1, K], F32),         # sum(embedding**2, axis=0)
        ("ET", [K, D], F32),          # embedding.T (gather table)
        ("emaT", [KSLC, D], F32),     # this core's slice of ema_w.T
        ("cs", [P, DC], F32),         # this core's cluster_size slice [p, jl]
    ]:
        din[name] = nc.dram_tensor(name, shape, dt, kind="ExternalInput")

    dout = {}
    for name, shape, dt in [
        ("qst", [NLOC, D], F32),
        ("idx", [NLOC, 1], I32),
        ("nemaT", [KSLC, D], F32),
        ("nembT", [KSLC, D], F32),
        ("ncs", [P, DC], F32),
        ("loss", [1, 1], F32),
    ]:
        dout[name] = nc.dram_tensor(name, shape, dt, kind="ExternalOutput")

    with tile.TileContext(nc) as tc:
        with (
            tc.tile_pool(name="const", bufs=1) as cp,
            tc.tile_pool(name="work", bufs=1) as wp,
            tc.tile_pool(name="ps", bufs=1, space="PSUM") as ps,
            tc.tile_pool(name="dram", bufs=1, space="DRAM") as dp,
        ):
            # ---------------- constants ----------------
            Eh_s, El_s = [], []
            for dc in range(DC):
                th = cp.tile([P, K], F16, tag=f"Eh{dc}", name=f"Eh{dc}")
                nc.sync.dma_start(th[:], din["Eh"][dc * P:(dc + 1) * P, :])
                Eh_s.append(th)
                tl = cp.tile([P, K], F16, tag=f"El{dc}", name=f"El{dc}")
                nc.sync.dma_start(tl[:], din["El"][dc * P:(dc + 1) * P, :])
                El_s.append(tl)
            esq_rep = cp.tile([P, K], F32, tag="esqrep")
            nc.sync.dma_start(esq_rep[:], din["esq"][:].to_broadcast([P, K]))
            ones8 = cp.tile([P, 8], F16, tag="ones8")
            nc.vector.memset(ones8[:], 1.0)
            ones16 = cp.tile([P, 1], F16, tag="ones16")
            nc.vector.memset(ones16[:], 1.0)
            ls_cols = cp.tile([P, TILES], F32, tag="lscols")

            # persistent PSUM bank for counts: col = g*KT + ct
            pcnt = ps.tile([P, NGRP * KT], F32, tag="pcnt")  # [128, 128]

            # DRAM scratch
            cc_in = dp.tile([K, CB + 1], F32, tag="ccin")
            rs_out = dp.tile([KSLC, CB + 1], F32, tag="rsout")
            ar2_in = dp.tile([1, 2], F32, tag="ar2in")
            ar2_out = dp.tile([1, 2], F32, tag="ar2out")

            oh_tiles = [None] * TILES
            x16_tiles = [None] * TILES

            def distance_chain(t):
                xh_c, xl_c = [], []
                for dc in range(DC):
                    th = wp.tile([P, P], F16, tag=f"xh{dc}", bufs=3,
                                 name=f"xh{dc}")
                    nc.sync.dma_start(
                        th[:], din["xh"][dc * P:(dc + 1) * P, t * P:(t + 1) * P])
                    xh_c.append(th)
                    tl = wp.tile([P, P], F16, tag=f"xl{dc}", bufs=3,
                                 name=f"xl{dc}")
                    nc.sync.dma_start(
                        tl[:], din["xl"][dc * P:(dc + 1) * P, t * P:(t + 1) * P])
                    xl_c.append(tl)
                x16t = wp.tile([P, D], F16, tag="x16t", bufs=GRP + 1)
                nc.sync.dma_start(x16t[:], din["x16"][t * P:(t + 1) * P, :])
                x16_tiles[t] = x16t

                oh = wp.tile([P, K], F16, tag="oh", bufs=GRP)
                oh_tiles[t] = oh
                m8h = []
                dhs = []
                for half in range(2):
                    d_h = wp.tile([P, HALF], F32, tag="dh", bufs=3, name="dh")
                    dhs.append(d_h)
                    pq = [ps.tile([P, CB], F32, tag="pq", bufs=4, name="pq")
                          for _ in range(4)]
                    for dc in range(DC):
                        for term, (xs, Es) in enumerate(
                                [(xh_c, Eh_s), (xh_c, El_s), (xl_c, Eh_s)]):
                            for q in range(4):
                                cb = half * 4 + q
                                nc.tensor.matmul(
                                    pq[q][:],
                                    xs[dc][:],
                                    Es[dc][:, cb * CB:(cb + 1) * CB],
                                    start=(dc == 0 and term == 0),
                                    stop=(dc == DC - 1 and term == 2),
                                    skip_group_check=True,
                                )
                    for q in range(4):
                        cb = half * 4 + q
                        lsl = slice(q * CB, (q + 1) * CB)
                        gsl = slice(cb * CB, (cb + 1) * CB)
                        nc.scalar.copy(d_h[:, lsl], pq[q][:])
                        nc.vector.tensor_tensor(
                            out=d_h[:, lsl], in0=d_h[:, lsl],
                            in1=esq_rep[:, gsl], op=mybir.AluOpType.subtract)
                    m8 = wp.tile([P, 8], F32, tag="m8h", bufs=4, name="m8h")
                    nc.vector.max(out=m8[:], in_=d_h[:])
                    m8h.append(m8)

                mg = wp.tile([P, 1], F32, tag="mg", bufs=2)
                nc.vector.tensor_tensor(
                    out=mg[:], in0=m8h[0][:, 0:1], in1=m8h[1][:, 0:1],
                    op=mybir.AluOpType.max)
                for half in range(2):
                    nc.vector.tensor_scalar(
                        out=oh[:, half * HALF:(half + 1) * HALF],
                        in0=dhs[half][:], scalar1=mg[:, 0:1], scalar2=None,
                        op0=mybir.AluOpType.is_equal)
                mi8 = wp.tile([P, 8], U32, tag="mi8", bufs=2)
                nc.vector.max_index(mi8[:], ones8[:], oh[:])
                idx32 = wp.tile([P, 1], I32, tag="idx32", bufs=2)
                nc.vector.tensor_copy(idx32[:], mi8[:, 0:1])
                nc.sync.dma_start(dout["idx"][t * P:(t + 1) * P, :], idx32[:])

                q_g = wp.tile([P, D], F32, tag="qg", bufs=2)
                nc.gpsimd.indirect_dma_start(
                    out=q_g[:], out_offset=None, in_=din["ET"][:],
                    in_offset=bass.IndirectOffsetOnAxis(ap=idx32[:, 0:1], axis=0))
                diff = wp.tile([P, D], F32, tag="diff", bufs=2)
                nc.vector.tensor_tensor(
                    out=diff[:], in0=q_g[:], in1=x16t[:],
                    op=mybir.AluOpType.subtract)
                # loss term: Square(diff) accumulated per-partition; the
                # elementwise result is written over q_g (dead).
                nc.scalar.activation(
                    q_g[:], diff[:], mybir.ActivationFunctionType.Square,
                    accum_out=ls_cols[:, t:t + 1])
                # straight-through output, in place over diff
                nc.vector.tensor_tensor(
                    out=diff[:], in0=x16t[:], in1=diff[:],
                    op=mybir.AluOpType.add)
                nc.sync.dma_start(dout["qst"][t * P:(t + 1) * P, :], diff[:])

            def embed_group(g):
                for ct in range(KT):
                    pe_ = ps.tile([P, CB], F32, tag="pe", bufs=2, name="pe")
                    for tl_ in range(GRP):
                        t = g * GRP + tl_
                        nc.tensor.matmul(
                            pe_[:],
                            oh_tiles[t][:, ct * P:(ct + 1) * P],
                            x16_tiles[t][:],
                            start=(tl_ == 0), stop=(tl_ == GRP - 1),
                            skip_group_check=True)
                    for tl_ in range(GRP):
                        t = g * GRP + tl_
                        nc.tensor.matmul(
                            pcnt[:, g * KT + ct:g * KT + ct + 1],
                            oh_tiles[t][:, ct * P:(ct + 1) * P],
                            ones16[:],
                            start=(tl_ == 0), stop=(tl_ == GRP - 1),
                            skip_group_check=True)
                    es = wp.tile([P, CB], F32, tag="es", bufs=3, name="es")
                    nc.scalar.copy(es[:], pe_[:])
                    if g == 0:
                        nc.sync.dma_start(
                            cc_in[ct * P:(ct + 1) * P, 0:CB], es[:])
                    else:
                        nc.gpsimd.dma_start(
                            cc_in[ct * P:(ct + 1) * P, 0:CB], es[:],
                            accum_op=mybir.AluOpType.add)

            for g in range(NGRP):
                for tl_ in range(GRP):
                    distance_chain(g * GRP + tl_)
                embed_group(g)

            # ---------------- counts + loss ----------------
            counts_sb = wp.tile([P, NGRP * KT], F32, tag="cntsb")
            nc.scalar.copy(counts_sb[:], pcnt[:])
            for g in range(1, NGRP):
                nc.vector.tensor_tensor(
                    out=counts_sb[:, 0:KT], in0=counts_sb[:, 0:KT],
                    in1=counts_sb[:, g * KT:(g + 1) * KT],
                    op=mybir.AluOpType.add)
            cdst = cc_in[0:K, CB:CB + 1].rearrange("(ct p) one -> p (ct one)", p=P)
            nc.sync.dma_start(cdst, counts_sb[:, 0:KT])

            ls_red = wp.tile([P, 1], F32, tag="lsred")
            nc.vector.reduce_sum(ls_red[:], ls_cols[:], axis=mybir.AxisListType.X)
            ls_all = wp.tile([P, 1], F32, tag="lsall")
            nc.gpsimd.partition_all_reduce(
                ls_all[:], ls_red[:], channels=P, reduce_op=bass_isa.ReduceOp.add)
            nc.sync.dma_start(ar2_in[0:1, 0:1], ls_all[0:1, 0:1])

            # ---------------- collectives ----------------
            nc.gpsimd.collective_compute(
                "ReduceScatter", mybir.AluOpType.add,
                replica_groups=[list(range(NCORES))],
                ins=[cc_in[:].opt()], outs=[rs_out[:].opt()])

            emb_sl = []
            for jl in range(DC):
                tte = wp.tile([P, CB + 1], F32, tag="embsl", bufs=DC,
                              name=f"embsl{jl}")
                nc.sync.dma_start(tte[:], rs_out[jl * P:(jl + 1) * P, :])
                emb_sl.append(tte)
            cs_in = wp.tile([P, DC], F32, tag="csin")
            nc.sync.dma_start(cs_in[:], din["cs"][:])
            cs99 = wp.tile([P, DC], F32, tag="cs99")
            nc.vector.tensor_scalar_mul(cs99[:], cs_in[:], DECAY)
            ncs = wp.tile([P, DC], F32, tag="ncst")
            for jl in range(DC):
                nc.vector.scalar_tensor_tensor(
                    out=ncs[:, jl:jl + 1], in0=emb_sl[jl][:, CB:CB + 1],
                    scalar=1.0 - DECAY, in1=cs99[:, jl:jl + 1],
                    op0=mybir.AluOpType.mult, op1=mybir.AluOpType.add)
            nc.sync.dma_start(dout["ncs"][:], ncs[:])
            nsum_p = wp.tile([P, 1], F32, tag="nsump")
            nc.vector.reduce_sum(nsum_p[:], ncs[:], axis=mybir.AxisListType.X)
            nsum_a = wp.tile([P, 1], F32, tag="nsuma")
            nc.gpsimd.partition_all_reduce(
                nsum_a[:], nsum_p[:], channels=P, reduce_op=bass_isa.ReduceOp.add)
            nc.sync.dma_start(ar2_in[0:1, 1:2], nsum_a[0:1, 0:1])

            nc.gpsimd.collective_compute(
                "AllReduce", mybir.AluOpType.add,
                replica_groups=[list(range(NCORES))],
                ins=[ar2_in[:].opt()], outs=[ar2_out[:].opt()])

            # ---------------- EMA update on this core's slice ----------------
            nl_rep = wp.tile([P, 2], F32, tag="nlrep")
            nc.sync.dma_start(nl_rep[:], ar2_out[:].to_broadcast([P, 2]))

            lossv = wp.tile([P, 1], F32, tag="lossv")
            nc.vector.tensor_scalar_mul(
                lossv[:], nl_rep[:, 0:1], COMMITMENT_COST / (N * D))
            nc.sync.dma_start(dout["loss"][:], lossv[0:1, 0:1])

            denom = wp.tile([P, 1], F32, tag="denom")
            nc.vector.tensor_scalar_add(denom[:], nl_rep[:, 1:2], K * EPS)
            rec = wp.tile([P, 1], F32, tag="rec")
            nc.vector.reciprocal(rec[:], denom[:])
            ffac = wp.tile([P, 1], F32, tag="ffac")
            nc.vector.tensor_tensor(
                out=ffac[:], in0=nl_rep[:, 1:2], in1=rec[:],
                op=mybir.AluOpType.mult)
            sm = wp.tile([P, DC], F32, tag="sm")
            nc.vector.tensor_scalar_add(sm[:], ncs[:], EPS)
            nc.vector.tensor_scalar(
                out=sm[:], in0=sm[:], scalar1=ffac[:, 0:1], scalar2=None,
                op0=mybir.AluOpType.mult)
            rsm = wp.tile([P, DC], F32, tag="rsm")
            nc.vector.reciprocal(rsm[:], sm[:])

            for jl in range(DC):
                emat = wp.tile([P, D], F32, tag="qg", bufs=2, name="emat")
                nc.sync.dma_start(emat[:], din["emaT"][jl * P:(jl + 1) * P, :])
                ema99 = wp.tile([P, D], F32, tag="diff", bufs=2, name="ema99")
                nc.vector.tensor_scalar_mul(ema99[:], emat[:], DECAY)
                nema = wp.tile([P, D], F32, tag="es", bufs=3, name="nema")
                nc.vector.scalar_tensor_tensor(
                    out=nema[:], in0=emb_sl[jl][:, 0:CB], scalar=1.0 - DECAY,
                    in1=ema99[:], op0=mybir.AluOpType.mult,
                    op1=mybir.AluOpType.add)
                nc.sync.dma_start(dout["nemaT"][jl * P:(jl + 1) * P, :], nema[:])
                # new embedding, reusing the ema99 slot
                nc.vector.tensor_scalar(
                    out=ema99[:], in0=nema[:], scalar1=rsm[:, jl:jl + 1],
                    scalar2=None, op0=mybir.AluOpType.mult)
                nc.sync.dma_start(dout["nembT"][jl * P:(jl + 1) * P, :], ema99[:])

    nc.compile()
    return nc


def _get_nc():
    if "nc" not in _CACHE:
        _CACHE["nc"] = _build_nc()
    return _CACHE["nc"]


def _prep_in_maps(x, embedding, cluster_size, ema_w):
    x = np.ascontiguousarray(np.asarray(x, dtype=np.float32))
    E = np.ascontiguousarray(np.asarray(embedding, dtype=np.float32))
    cs = np.asarray(cluster_size, dtype=np.float32)
    ema = np.asarray(ema_w, dtype=np.float32)

    Ea = 2.0 * E
    Eh = Ea.astype(np.float16)
    El = (Ea - Eh.astype(np.float32)).astype(np.float16)
    esq = (E.astype(np.float64) ** 2).sum(0).astype(np.float32)[None, :]
    ET = np.ascontiguousarray(E.T)
    emaT = np.ascontiguousarray(ema.T)          # [K, D]
    cs_r = cs.reshape(NCORES, DC, P)            # [c, jl, p]

    xT = np.ascontiguousarray(x.T)              # [D, N]
    xh_full = xT.astype(np.float16)
    xl_full = (xT - xh_full.astype(np.float32)).astype(np.float16)

    in_maps = []
    for c in range(NCORES):
        tok = slice(c * NLOC, (c + 1) * NLOC)
        kk = slice(c * KSLC, (c + 1) * KSLC)
        in_maps.append(dict(
            xh=np.ascontiguousarray(xh_full[:, tok]),
            xl=np.ascontiguousarray(xl_full[:, tok]),
            x16=x[tok].astype(np.float16),
            Eh=Eh, El=El, esq=esq, ET=ET,
            emaT=np.ascontiguousarray(emaT[kk]),
            cs=np.ascontiguousarray(cs_r[c].T),  # [p, jl]
        ))
    return in_maps


def _assemble(results):
    qst = np.concatenate([r["qst"] for r in results], axis=0)
    idx = np.concatenate([r["idx"] for r in results], axis=0).astype(np.int32)
    loss = np.float32(results[0]["loss"][0, 0])
    nembT = np.concatenate([r["nembT"] for r in results], axis=0)  # [K, D]
    nemaT = np.concatenate([r["nemaT"] for r in results], axis=0)
    new_embedding = np.ascontiguousarray(nembT.T)
    new_ema_w = np.ascontiguousarray(nemaT.T)
    ncs = np.concatenate(
        [r["ncs"].T.reshape(KSLC) for r in results], axis=0)
    return (qst, loss, idx, new_embedding, ncs, new_ema_w)


def kernel(x, embedding, cluster_size, ema_w):
    nc = _get_nc()
    in_maps = _prep_in_maps(x, embedding, cluster_size, ema_w)
    res = run_bass_kernel_spmd(nc, in_maps, core_ids=list(range(NCORES)))
    return _assemble(res.results)
